# revision 1
# baseline (speedup 1.0000x reference)
"""GAT (3-layer, 4-head) Trainium2 kernel, 8-core SPMD.

Sharding: nodes are partitioned into 8 contiguous ranges of 1250 (graph
parallel).  Edges (self-loops included) are sorted by destination and owned
by the destination's core.  Weights are replicated.  Each core computes
x1 = relu(x @ W_ah + b) and h_l = x1 @ W_l for its node slice; a fused
per-node table [h0|as0|h1|as1|h2|as2] (3*772 bf16 columns) is AllGathered,
and each core gathers source rows for its edges with indirect DMA (halo
exchange by gather) — one gather serves all three layers.  Segment softmax
is computed without the max subtraction (alpha is shift-invariant; logits
here are O(1) so exp cannot overflow).  Per-destination aggregation runs on
the tensor engine with 0/1 selection matrices built by is_equal against an
iota row, accumulating in PSUM across 128-edge chunks; 1/sum normalization
is applied per-partition at eviction.
"""

import sys

sys.path.insert(0, "/opt/trn_rl_repo")

import numpy as np

import concourse.bass as bass
import concourse.bacc as bacc
import concourse.mybir as mybir
import concourse.tile as tile
from concourse.bass_utils import run_bass_kernel_spmd

N, E, D, H, C = 10000, 100000, 768, 4, 192
NEG_SLOPE = 0.2
NCORES = 8
NPC = N // NCORES          # nodes per core
P = 128
NTILES = (NPC + P - 1) // P  # dst tiles per core (last one is 98 rows)
FEXT = D + H               # per-layer row block: [h (768) | a_src.h (4)]
FROW = 3 * FEXT            # fused gather row (2316)
KT = D // P                # 6 contraction tiles for D
HG = 5                     # chunks per gather group
# AllGather split: thirds of each core's row range; (j0, j1, out_base)
_T1, _T2 = 417, 834
AG_RANGES = ((0, _T1, 0), (_T1, _T2, NCORES * _T1),
             (_T2, NPC, NCORES * _T2))
F32 = mybir.dt.float32
BF16 = mybir.dt.bfloat16
I32 = mybir.dt.int32
GDT = BF16                 # dtype of gathered tables / selection matrices

_cache = {}


def _prep_edges(edge_index):
    """Sort edges by destination, partition by dst core/tile; within each
    tile group edges by the THIRD of the node range their source falls in
    (so each 128-edge chunk gathers from a single AllGather shard table),
    padding each (tile, third) group to a multiple of 128 edges, with
    chunk counts made uniform across cores for SPMD.

    Returns (esrc [8,128,TOT_M] int32  -- remapped per-table row index,
             edst [8,128,TOT_M] f32, M3_list tuple of per-tile 3-tuples)."""
    src = np.concatenate([edge_index[0], np.arange(N, dtype=np.int32)])
    dst = np.concatenate([edge_index[1], np.arange(N, dtype=np.int32)])
    order = np.argsort(dst, kind="stable")
    src_s = src[order].astype(np.int64)
    dst_s = dst[order].astype(np.int64)

    # source third + row index within that third's table
    sc = src_s // NPC
    sj = src_s % NPC
    sthird = np.zeros(len(src_s), dtype=np.int64)
    sidx = np.zeros(len(src_s), dtype=np.int64)
    for r, (j0, j1, _o) in enumerate(AG_RANGES):
        m = (sj >= j0) & (sj < j1)
        sthird[m] = r
        sidx[m] = sc[m] * (j1 - j0) + (sj[m] - j0)

    starts = np.empty((NCORES, NTILES), dtype=np.int64)
    ends = np.empty((NCORES, NTILES), dtype=np.int64)
    for c in range(NCORES):
        for t in range(NTILES):
            lo = c * NPC + t * P
            hi = min(c * NPC + (t + 1) * P, (c + 1) * NPC)
            starts[c, t] = np.searchsorted(dst_s, lo, side="left")
            ends[c, t] = np.searchsorted(dst_s, hi, side="left")

    # per (core, tile, third) counts
    cnt3 = np.zeros((NCORES, NTILES, 3), dtype=np.int64)
    for c in range(NCORES):
        for t in range(NTILES):
            th = sthird[starts[c, t] : ends[c, t]]
            for r in range(3):
                cnt3[c, t, r] = int((th == r).sum())
    M3_list = tuple(
        tuple(int(max(1 if r == 0 else 0, -(-cnt3[:, t, r].max() // P)))
              for r in range(3))
        for t in range(NTILES)
    )
    M_flat = [sum(m3) for m3 in M3_list]
    M_off = np.concatenate([[0], np.cumsum(M_flat)]).astype(int)
    TOT_M = int(M_off[-1])

    esrc = np.zeros((NCORES, P, TOT_M), dtype=np.int32)
    edst = np.full((NCORES, P, TOT_M), -1.0, dtype=np.float32)
    for c in range(NCORES):
        for t in range(NTILES):
            seg = slice(starts[c, t], ends[c, t])
            th = sthird[seg]
            si = sidx[seg]
            dl = (dst_s[seg] - c * NPC - t * P).astype(np.float32)
            col = int(M_off[t])
            for r in range(3):
                Mr = M3_list[t][r]
                if Mr == 0:
                    continue
                sel = th == r
                n = int(sel.sum())
                sv = np.zeros(Mr * P, dtype=np.int32)
                dv = np.full(Mr * P, -1.0, dtype=np.float32)
                sv[:n] = si[sel]
                dv[:n] = dl[sel]
                esrc[c][:, col : col + Mr] = sv.reshape(Mr, P).T
                edst[c][:, col : col + Mr] = dv.reshape(Mr, P).T
                col += Mr
    return esrc, edst, M3_list


def _build(M3_list, has_b, has_bout, dbg=False, reps=1, no_cc=False,
           phases=(1, 3, 4)):
    M_list = [sum(m3) for m3 in M3_list]
    TOT_M = sum(M_list)
    M_off = np.concatenate([[0], np.cumsum(M_list)]).astype(int)
    # flat per-tile chunk -> shard-table id
    rtab = [[r for r in range(3) for _ in range(M3_list[t][r])]
            for t in range(NTILES)]

    nc = bacc.Bacc("TRN2", target_bir_lowering=False, debug=False,
                   num_devices=NCORES)
    if dbg:
        t_dbg_x1 = nc.dram_tensor("dbg_x1", [D, NPC], F32,
                                  kind="ExternalOutput")
        t_dbg_ne = nc.dram_tensor("dbg_ne", [P, NTILES * 3 * D], F32,
                                  kind="ExternalOutput")

    t_xT = nc.dram_tensor("xT", [D, NPC], F32, kind="ExternalInput")
    t_wah = nc.dram_tensor("W_ah", [D, D], F32, kind="ExternalInput")
    t_bah = nc.dram_tensor("bah", [P, KT], F32, kind="ExternalInput")
    t_wall = nc.dram_tensor("W_all", [D, 3 * D], F32, kind="ExternalInput")
    t_waa = nc.dram_tensor("Waa", [D, 24], F32, kind="ExternalInput")
    t_esrc = nc.dram_tensor("esrc", [P, TOT_M], I32, kind="ExternalInput")
    t_edst = nc.dram_tensor("edst", [P, TOT_M], F32, kind="ExternalInput")
    t_iota = nc.dram_tensor("iota", [P, P], F32, kind="ExternalInput")
    t_ident = nc.dram_tensor("ident", [P, P], F32, kind="ExternalInput")
    if has_b:
        t_bcat = nc.dram_tensor("bcat_bc", [P, 3 * D], F32,
                                kind="ExternalInput")
    t_wout = nc.dram_tensor("W_out", [3 * D, D], F32, kind="ExternalInput")
    if has_bout:
        t_bout = nc.dram_tensor("bout_bc", [P, D], F32, kind="ExternalInput")
    t_out = nc.dram_tensor("out_slice", [NPC, D], F32, kind="ExternalOutput")

    mm = nc.tensor.matmul
    eq = mybir.AluOpType.is_equal
    ADD = mybir.AluOpType.add
    MUL = mybir.AluOpType.mult
    AF = mybir.ActivationFunctionType

    with tile.TileContext(nc) as tc:
        with (
            tc.tile_pool(name="consts", bufs=1) as cp,
            tc.tile_pool(name="dram", bufs=1, space="DRAM") as dp,
        ):
            for rep in range(reps):
                iota_sb = cp.tile([P, P], F32)
                nc.sync.dma_start(iota_sb[:], t_iota.ap())
                ident_sb = cp.tile([P, P], F32)
                nc.sync.dma_start(ident_sb[:], t_ident.ap())
                ident_bf = cp.tile([P, P], GDT)
                nc.vector.tensor_copy(ident_bf[:], ident_sb[:])
                esrc_sb = cp.tile([P, TOT_M], I32)
                nc.sync.dma_start(esrc_sb[:], t_esrc.ap())
                edst_sb = cp.tile([P, TOT_M], F32)
                nc.sync.dma_start(edst_sb[:], t_edst.ap())
                bah_sb = cp.tile([P, KT], F32)
                nc.sync.dma_start(bah_sb[:], t_bah.ap())
                if has_b:
                    bcat_sb = cp.tile([P, 3 * D], F32)
                    nc.sync.dma_start(bcat_sb[:], t_bcat.ap())
                if has_bout:
                    bout_sb = cp.tile([P, D], F32)
                    nc.sync.dma_start(bout_sb[:], t_bout.ap())
                asad_sb = cp.tile([P, NTILES * 24], F32)
                nc.gpsimd.memset(asad_sb[:], 0.0)
                asad_bf = cp.tile([P, NTILES * 24], GDT)
                nodeemb = cp.tile([P, NTILES * 3 * D], F32)

                hext = [dp.tile([j1 - j0, FROW], GDT, name=f"hext{r}")
                        for r, (j0, j1, _o) in enumerate(AG_RANGES)]
                hfull = [dp.tile([NCORES * (j1 - j0), FROW], GDT,
                                 addr_space="Shared", name=f"hfull{r}")
                         for r, (j0, j1, _o) in enumerate(AG_RANGES)]

                # ---------------- phase 1: dense matmuls ----------------
                with tc.tile_pool(name="ph1x1", bufs=1) as xp1:
                    x1sb = [xp1.tile([P, NPC], F32, name=f"x1_{k}")
                            for k in range(KT)]
                    with (
                        tc.tile_pool(name="ph1a", bufs=1) as wpa,
                        tc.tile_pool(name="px1", bufs=1, space="PSUM") as px1,
                    ):
                        xsb, wah = [], []
                        for k in range(KT):
                            xk = wpa.tile([P, NPC], F32, name=f"xsb{k}")
                            nc.sync.dma_start(
                                xk[:], t_xT.ap()[k * P : (k + 1) * P, :])
                            xsb.append(xk)
                            wk = wpa.tile([P, D], F32, name=f"wah{k}")
                            nc.sync.dma_start(
                                wk[:], t_wah.ap()[k * P : (k + 1) * P, :])
                            wah.append(wk)

                        # x1T = relu(x @ W_ah + b), feat-major [768, 1250]
                        # k-outer so matmuls start as soon as xsb[k] lands
                        nch = [(0, 512), (512, 512), (1024, NPC - 1024)]
                        for jp in range(KT // 2):
                            pss = [px1.tile([P, 512], F32, space="PSUM",
                                            name=f"psx1_{q}")
                                   for q in range(6)]
                            for k in range(KT):
                                for j2 in range(2):
                                    j = jp * 2 + j2
                                    for n, (n0, nw) in enumerate(nch):
                                        mm(out=pss[j2 * 3 + n][:, :nw],
                                           lhsT=wah[k][:, j * P
                                                       : (j + 1) * P],
                                           rhs=xsb[k][:, n0 : n0 + nw],
                                           start=(k == 0),
                                           stop=(k == KT - 1))
                            for j2 in range(2):
                                j = jp * 2 + j2
                                for n, (n0, nw) in enumerate(nch):
                                    nc.scalar.activation(
                                        x1sb[j][:, n0 : n0 + nw],
                                        pss[j2 * 3 + n][:, :nw], AF.Relu,
                                        bias=bah_sb[:, j : j + 1])
                    if dbg:
                        for k in range(KT):
                            nc.sync.dma_start(
                                t_dbg_x1.ap()[k * P : (k + 1) * P, :],
                                x1sb[k][:])

                    with (
                        tc.tile_pool(name="ph1b", bufs=1) as wpb,
                        tc.tile_pool(name="ph1st", bufs=3) as sp1,
                        tc.tile_pool(name="ph", bufs=1, space="PSUM") as phh,
                    ):
                        wall, waa = [], []
                        for k in range(KT):
                            ak = wpb.tile([P, 3 * D], F32, name=f"wall{k}")
                            nc.sync.dma_start(
                                ak[:], t_wall.ap()[k * P : (k + 1) * P, :])
                            wall.append(ak)
                            bk = wpb.tile([P, 24], F32, name=f"waa{k}")
                            nc.sync.dma_start(
                                bk[:], t_waa.ap()[k * P : (k + 1) * P, :])
                            waa.append(bk)

                        # h_l = x1 @ W_l for l=0..2 and asad = x1 @ Waa,
                        # k-outer so each lhsT loads once per (t, k).
                        for t in range(NTILES):
                            m0 = t * P
                            mw = min(P, NPC - m0)
                            hps = [phh.tile([P, 384], F32, space="PSUM",
                                            name=f"psh{q}")
                                   for q in range(6)]
                            aps = phh.tile([P, 24], F32, space="PSUM",
                                           name="psasad")
                            for k in range(KT):
                                for q in range(6):
                                    mm(out=hps[q][:mw, :],
                                       lhsT=x1sb[k][:, m0 : m0 + mw],
                                       rhs=wall[k][:, q * 384 : q * 384 + 384],
                                       start=(k == 0), stop=(k == KT - 1))
                                mm(out=aps[:mw, :],
                                   lhsT=x1sb[k][:, m0 : m0 + mw],
                                   rhs=waa[k][:],
                                   start=(k == 0), stop=(k == KT - 1))
                            nc.vector.tensor_copy(
                                asad_sb[:mw, t * 24 : t * 24 + 24],
                                aps[:mw, :])
                            st = sp1.tile([P, FROW], GDT, name="hstage")
                            for l in range(3):
                                for hh in range(2):
                                    nc.vector.tensor_copy(
                                        st[:mw, l * FEXT + hh * 384
                                           : l * FEXT + hh * 384 + 384],
                                        hps[l * 2 + hh][:mw, :])
                                nc.vector.tensor_copy(
                                    st[:mw, l * FEXT + D : l * FEXT + D + H],
                                    asad_sb[:mw, t * 24 + l * 8
                                            : t * 24 + l * 8 + 4])
                            for r, (j0, j1, _o) in enumerate(AG_RANGES):
                                lo = max(m0, j0)
                                hi = min(m0 + mw, j1)
                                if lo < hi:
                                    nc.sync.dma_start(
                                        hext[r][lo - j0 : hi - j0, :],
                                        st[lo - m0 : hi - m0, :])
                        if not no_cc:
                            for r in range(3):
                                nc.gpsimd.collective_compute(
                                    "AllGather", mybir.AluOpType.bypass,
                                    replica_groups=[list(range(NCORES))],
                                    ins=[hext[r][:].opt()],
                                    outs=[hfull[r][:].opt()],
                                )

                nc.vector.tensor_copy(asad_bf[:], asad_sb[:])

                # -------------- phase 3: gather + aggregate --------------
                if 3 in phases:
                    with (
                        tc.tile_pool(name="hgp", bufs=3) as hgp,
                        tc.tile_pool(name="sgp", bufs=3) as sgp,
                        tc.tile_pool(name="smallp", bufs=4) as smp,
                        tc.tile_pool(name="psA", bufs=1, space="PSUM") as psA,
                        tc.tile_pool(name="psT", bufs=1, space="PSUM") as psT,
                        tc.tile_pool(name="psB", bufs=1, space="PSUM") as psB,
                    ):
                        for t in range(NTILES):
                            Mt = M_list[t]
                            Moff = int(M_off[t])
                            aggs = [psA.tile([P, FEXT], F32, space="PSUM",
                                             name=f"agg{l}")
                                    for l in range(3)]
                            done = 0
                            for c0 in range(0, Mt, HG):
                                g = min(HG, Mt - c0)
                                hg = hgp.tile([P, HG * FROW], GDT, name="hg")
                                for m in range(g):
                                    nc.gpsimd.indirect_dma_start(
                                        out=hg[:, m * FROW : (m + 1) * FROW],
                                        out_offset=None,
                                        in_=hfull[rtab[t][c0 + m]][:],
                                        in_offset=bass.IndirectOffsetOnAxis(
                                            ap=esrc_sb[:, Moff + c0 + m
                                                       : Moff + c0 + m + 1],
                                            axis=0),
                                    )
                                s_all = sgp.tile([P, HG * P], GDT,
                                                 name="s_all")
                                nc.vector.tensor_tensor(
                                    out=s_all[:, : g * P].rearrange(
                                        "p (g n) -> p g n", n=P),
                                    in0=iota_sb[:].unsqueeze(1).to_broadcast(
                                        [P, g, P]),
                                    in1=edst_sb[:, Moff + c0 : Moff + c0 + g]
                                    .unsqueeze(2).to_broadcast([P, g, P]),
                                    op=eq)
                                st_all = sgp.tile([P, HG * P], GDT,
                                                  name="st_all")
                                for q0 in range(0, g, 2):
                                    qn = min(2, g - q0)
                                    stp = psT.tile([P, 256], GDT,
                                                   space="PSUM", name="stps")
                                    for j in range(qn):
                                        nc.tensor.transpose(
                                            out=stp[:, j * P : (j + 1) * P],
                                            in_=s_all[:, (q0 + j) * P
                                                      : (q0 + j + 1) * P],
                                            identity=ident_bf[:])
                                    nc.vector.tensor_copy(
                                        st_all[:, q0 * P : (q0 + qn) * P],
                                        stp[:, : qn * P])
                                adp = psB.tile([P, 3 * HG * H], F32,
                                               space="PSUM", name="adp")
                                for l in range(3):
                                    ad_col = t * 24 + l * 8 + 4
                                    for m in range(g):
                                        mm(out=adp[:, (l * HG + m) * H
                                                   : (l * HG + m + 1) * H],
                                           lhsT=st_all[:, m * P
                                                       : (m + 1) * P],
                                           rhs=asad_bf[:, ad_col
                                                       : ad_col + H],
                                           start=True, stop=True)
                                hg4 = hg[:].rearrange(
                                    "p (g l f) -> p g l f", l=3, f=FEXT)
                                t_all = smp.tile([P, 3 * HG * H], F32,
                                                 name="t_all")
                                t4 = t_all[:].rearrange(
                                    "p (l m f) -> p l m f", l=3, f=H)
                                nc.vector.tensor_tensor(
                                    out=t4,
                                    in0=hg4[:, :, :, D : D + H].transpose(
                                        [0, 2, 1, 3]),
                                    in1=adp[:].rearrange(
                                        "p (l m f) -> p l m f", l=3, f=H),
                                    op=ADD)
                                u_all = smp.tile([P, 3 * HG * H], F32,
                                                 name="u_all")
                                nc.vector.tensor_scalar_mul(
                                    u_all[:], t_all[:], NEG_SLOPE)
                                nc.vector.tensor_tensor(
                                    out=t_all[:], in0=t_all[:], in1=u_all[:],
                                    op=mybir.AluOpType.max)
                                p_all = smp.tile([P, 3 * HG * H], F32,
                                                 name="p_all")
                                nc.scalar.activation(p_all[:], t_all[:],
                                                     AF.Exp)
                                p4 = p_all[:].rearrange(
                                    "p (l m f) -> p l m f", l=3, f=H)
                                nc.vector.tensor_copy(
                                    hg4[:, :, :, D : D + H],
                                    p4.transpose([0, 2, 1, 3]))
                                for m in range(g):
                                    for l in range(3):
                                        base = m * FROW + l * FEXT
                                        for h in range(H):
                                            pc = (l * HG + m) * H + h
                                            nc.vector.tensor_scalar(
                                                out=hg[:, base + h * C
                                                       : base + (h + 1) * C],
                                                in0=hg[:, base + h * C
                                                       : base + (h + 1) * C],
                                                scalar1=p_all[:, pc : pc + 1],
                                                scalar2=None, op0=MUL)
                                for m in range(g):
                                    gi = done + m
                                    for l in range(3):
                                        base = m * FROW + l * FEXT
                                        mm(out=aggs[l][:, 0:512],
                                           lhsT=s_all[:, m * P : (m + 1) * P],
                                           rhs=hg[:, base : base + 512],
                                           start=(gi == 0),
                                           stop=(gi == Mt - 1))
                                        mm(out=aggs[l][:, 512:FEXT],
                                           lhsT=s_all[:, m * P : (m + 1) * P],
                                           rhs=hg[:, base + 512
                                                  : base + FEXT],
                                           start=(gi == 0),
                                           stop=(gi == Mt - 1))
                                done += g
                            for l in range(3):
                                invs = smp.tile([P, H], F32, name="invs")
                                nc.vector.tensor_scalar_add(
                                    invs[:], aggs[l][:, D : D + H], 1e-16)
                                nc.vector.reciprocal(invs[:], invs[:])
                                ne0 = t * 3 * D + l * D
                                for h in range(H):
                                    nc.vector.tensor_scalar(
                                        out=nodeemb[:, ne0 + h * C
                                                    : ne0 + (h + 1) * C],
                                        in0=aggs[l][:, h * C : (h + 1) * C],
                                        scalar1=invs[:, h : h + 1],
                                        scalar2=None, op0=MUL)
                                if has_b:
                                    nc.vector.tensor_tensor(
                                        out=nodeemb[:, ne0 : ne0 + D],
                                        in0=nodeemb[:, ne0 : ne0 + D],
                                        in1=bcat_sb[:, l * D : (l + 1) * D],
                                        op=ADD)
                                nc.scalar.activation(
                                    nodeemb[:, ne0 : ne0 + D],
                                    nodeemb[:, ne0 : ne0 + D], AF.Relu)

                # -------------- phase 4: output projection --------------
                if 4 in phases:
                    with (
                        tc.tile_pool(name="ph4w", bufs=1) as wp4,
                        tc.tile_pool(name="ph4s", bufs=2) as sp4,
                        tc.tile_pool(name="pT4", bufs=2, space="PSUM") as pT4,
                        tc.tile_pool(name="pO4", bufs=1, space="PSUM") as pO4,
                    ):
                        wout = []
                        for k in range(3 * KT):
                            wk = wp4.tile([P, D], F32, name=f"wout{k}")
                            nc.sync.dma_start(
                                wk[:], t_wout.ap()[k * P : (k + 1) * P, :])
                            wout.append(wk)
                        for t in range(NTILES):
                            mw = min(P, NPC - t * P)
                            tsbs = []
                            for hf in range(2):
                                tp = pT4.tile([P, 9 * P], F32, space="PSUM",
                                              name="tps")
                                for k9 in range(9):
                                    k = hf * 9 + k9
                                    nc.tensor.transpose(
                                        out=tp[:, k9 * P : (k9 + 1) * P],
                                        in_=nodeemb[:, t * 3 * D + k * P
                                                    : t * 3 * D
                                                    + (k + 1) * P],
                                        identity=ident_sb[:])
                                tsb = sp4.tile([P, 9 * P], F32, name="tsb",
                                               bufs=4)
                                nc.vector.tensor_copy(tsb[:], tp[:])
                                tsbs.append(tsb)
                            op = pO4.tile([P, D], F32, space="PSUM",
                                          name="ops")
                            for k in range(3 * KT):
                                lh = tsbs[k // 9][:, (k % 9) * P
                                                  : (k % 9 + 1) * P]
                                for (n0, nw) in [(0, 512), (512, 256)]:
                                    mm(out=op[:, n0 : n0 + nw],
                                       lhsT=lh,
                                       rhs=wout[k][:, n0 : n0 + nw],
                                       start=(k == 0), stop=(k == 3 * KT - 1))
                            stg = sp4.tile([P, D], F32, name="ostage")
                            if has_bout:
                                nc.vector.tensor_tensor(out=stg[:], in0=op[:],
                                                        in1=bout_sb[:],
                                                        op=ADD)
                                nc.scalar.activation(stg[:], stg[:], AF.Relu)
                            else:
                                nc.scalar.activation(stg[:], op[:], AF.Relu)
                            nc.sync.dma_start(
                                t_out.ap()[t * P : t * P + mw, :],
                                stg[:mw, :])

                if dbg:
                    nc.sync.dma_start(t_dbg_ne.ap()[:, :], nodeemb[:])

    nc.compile()
    return nc


def _host_prep(inputs):
    x = np.asarray(inputs["x"], dtype=np.float32)
    edge_index = np.asarray(inputs["edge_index"], dtype=np.int32)
    W_ah = np.asarray(inputs["W_ah"], dtype=np.float32)
    b_ah = np.asarray(inputs["b_ah"], dtype=np.float32)
    W_out = np.asarray(inputs["W_out"], dtype=np.float32)
    b_out = np.asarray(inputs["b_out"], dtype=np.float32)
    Ws = [np.asarray(inputs[f"W{i}"], dtype=np.float32) for i in range(3)]
    asrcs = [np.asarray(inputs[f"a_src{i}"], dtype=np.float32)
             for i in range(3)]
    adsts = [np.asarray(inputs[f"a_dst{i}"], dtype=np.float32)
             for i in range(3)]
    bs = [np.asarray(inputs[f"b{i}"], dtype=np.float32) for i in range(3)]

    esrc, edst, M3_list = _prep_edges(edge_index)

    W_all = np.ascontiguousarray(np.concatenate(Ws, axis=1))
    Waa = np.zeros((D, 24), dtype=np.float32)
    for l in range(3):
        Amat_s = np.zeros((D, H), dtype=np.float32)
        Amat_d = np.zeros((D, H), dtype=np.float32)
        for h in range(H):
            Amat_s[h * C : (h + 1) * C, h] = asrcs[l][h]
            Amat_d[h * C : (h + 1) * C, h] = adsts[l][h]
        Waa[:, l * 8 : l * 8 + 4] = Ws[l] @ Amat_s
        Waa[:, l * 8 + 4 : l * 8 + 8] = Ws[l] @ Amat_d
    Waa = np.ascontiguousarray(Waa)

    bah2 = np.ascontiguousarray(b_ah.reshape(KT, P).T)
    bcat = np.concatenate(bs)
    has_b = bool(np.any(bcat))
    has_bout = bool(np.any(b_out))

    iota = np.broadcast_to(np.arange(P, dtype=np.float32), (P, P)).copy()
    ident = np.eye(P, dtype=np.float32)

    shared = {
        "W_ah": W_ah, "bah": bah2, "W_all": W_all, "Waa": Waa,
        "iota": iota, "ident": ident, "W_out": np.ascontiguousarray(W_out),
    }
    if has_b:
        shared["bcat_bc"] = np.broadcast_to(bcat, (P, 3 * D)).copy()
    if has_bout:
        shared["bout_bc"] = np.broadcast_to(b_out, (P, D)).copy()

    in_maps = []
    for c in range(NCORES):
        m = dict(shared)
        m["xT"] = np.ascontiguousarray(x[c * NPC : (c + 1) * NPC].T)
        m["esrc"] = np.ascontiguousarray(esrc[c])
        m["edst"] = np.ascontiguousarray(edst[c])
        in_maps.append(m)
    return in_maps, M3_list, has_b, has_bout


def run(inputs, trace=False, dbg=False, reps=1):
    in_maps, M3_list, has_b, has_bout = _host_prep(inputs)
    key = (M3_list, has_b, has_bout, dbg, reps)
    if key not in _cache:
        _cache[key] = _build(M3_list, has_b, has_bout, dbg=dbg,
                             reps=reps)
    nc = _cache[key]
    res = run_bass_kernel_spmd(nc, in_maps, core_ids=list(range(NCORES)),
                               trace=trace)
    out = np.concatenate([res.results[c]["out_slice"]
                          for c in range(NCORES)], axis=0)
    return out, res


def kernel(**inputs) -> np.ndarray:
    out, _ = run(inputs, trace=False)
    return out



# revision 16
# speedup vs baseline: 3.1433x; 3.1433x over previous
"""GAT (3-layer, 4-head) Trainium2 kernel, 8-core SPMD.

Sharding: nodes partitioned into 8 contiguous ranges of 1250 (graph
parallel).  Edges (self-loops included) are sorted by destination and owned
by the destination's core.  Weights replicated.  Each core computes
x1 = relu(x @ W_ah + b) and h_l = x1 @ W_l for its node slice in bf16; a
fused per-node table [h0|as0|h1|as1|h2|as2|pad] (2432 bf16 columns, row
pitch 256B-aligned for dma_gather) is AllGathered once; each core then
gathers source rows for its edges with dma_gather (one instruction per
~6 chunks of 128 edges).  Segment softmax runs without max subtraction
(logits are O(0.1)).  Per-destination aggregation uses 0/1 selection
matrices (built by is_equal on DVE/Pool; the transposed selection comes
from a host-prepared transposed-dst table, no PE transpose) contracted on
the tensor engine, accumulating in PSUM across a tile's chunks; exp values
are written into the table's attention slots so the same matmul yields the
softmax denominators; eviction fuses 1/sum scaling + ReLU on the scalar
engine.  Alpha-scaling of gathered rows is split across DVE and Pool.
"""

import sys

sys.path.insert(0, "/opt/trn_rl_repo")

import numpy as np
import ml_dtypes

import concourse.bass as bass
import concourse.bacc as bacc
import concourse.mybir as mybir
import concourse.tile as tile
from concourse.bass_utils import run_bass_kernel_spmd

N, E, D, H, C = 10000, 100000, 768, 4, 192
NEG_SLOPE = 0.2
NCORES = 8
NPC = N // NCORES          # nodes per core
P = 128
NTILES = (NPC + P - 1) // P  # dst tiles per core (last one is 98 rows)
FEXT = D + H               # per-layer row block: [h (768) | a_src.h (4)]
FROW = 3 * FEXT            # useful row columns (2316)
FROWP = 2432               # padded row pitch (4864 B = 19*256)
KT = D // P                # 6 contraction tiles for D
HG = 6                     # chunks per dma_gather group
F32 = mybir.dt.float32
BF16 = mybir.dt.bfloat16
I16 = mybir.dt.int16
NPBF = ml_dtypes.bfloat16

_cache = {}


def _prep_edges(edge_index):
    """Sort edges by destination, partition into per-(core, dst-tile) chunks
    of 128 (padded; chunk counts uniform across cores for SPMD).

    Returns (esrc16 [8,128,TOT_M*8] int16 dma_gather index layout,
             edst   [8,128,TOT_M]  f32 local dst per edge slot (-1 pad),
             edstT  [8,128,TOT_M*128] bf16 same transposed (partition-
                    replicated, edge slot along free axis),
             M_list per-tile chunk counts)."""
    src = np.concatenate([edge_index[0], np.arange(N, dtype=np.int32)])
    dst = np.concatenate([edge_index[1], np.arange(N, dtype=np.int32)])
    order = np.argsort(dst, kind="stable")
    src_s = src[order].astype(np.int64)
    dst_s = dst[order].astype(np.int64)

    starts = np.empty((NCORES, NTILES), dtype=np.int64)
    ends = np.empty((NCORES, NTILES), dtype=np.int64)
    for c in range(NCORES):
        for t in range(NTILES):
            lo = c * NPC + t * P
            hi = min(c * NPC + (t + 1) * P, (c + 1) * NPC)
            starts[c, t] = np.searchsorted(dst_s, lo, side="left")
            ends[c, t] = np.searchsorted(dst_s, hi, side="left")
    cnt = ends - starts
    M_list = tuple(int(max(1, -(-cnt[:, t].max() // P))) for t in range(NTILES))
    M_off = np.concatenate([[0], np.cumsum(M_list)]).astype(int)
    TOT_M = int(M_off[-1])

    esrc16 = np.zeros((NCORES, P, TOT_M * 8), dtype=np.int16)
    edst = np.full((NCORES, P, TOT_M), -1.0, dtype=np.float32)
    edstT = np.empty((NCORES, P, TOT_M * P), dtype=NPBF)
    for c in range(NCORES):
        for t in range(NTILES):
            Mt = M_list[t]
            n = int(cnt[c, t])
            seg = slice(starts[c, t], ends[c, t])
            sv = np.zeros(Mt * P, dtype=np.int64)
            sv[:n] = src_s[seg]
            dv = np.full(Mt * P, -1.0, dtype=np.float32)
            dv[:n] = (dst_s[seg] - c * NPC - t * P).astype(np.float32)
            col = int(M_off[t])
            edst[c][:, col : col + Mt] = dv.reshape(Mt, P).T
            edstT[c][:, col * P : (col + Mt) * P] = np.broadcast_to(
                dv.astype(NPBF), (P, Mt * P))
            # dma_gather index layout: idx j lives at [j%16, base + j//16],
            # replicated across all eight 16-partition stripes (one per
            # gpsimd DSP core).
            for g0 in range(0, Mt, HG):
                gl = min(HG, Mt - g0)
                nidx = gl * P
                j = np.arange(nidx)
                vals = sv[g0 * P : g0 * P + nidx]
                base = (col + g0) * 8
                for s16 in range(0, P, 16):
                    esrc16[c][s16 + j % 16, base + j // 16] = vals.astype(
                        np.int16)
    return esrc16, edst, edstT, M_list


def _build(M_list, has_b, has_bout, dbg=False, reps=1, no_cc=False,
           phases=(1, 3, 4)):
    M_list = tuple(M_list)
    TOT_M = sum(M_list)
    M_off = np.concatenate([[0], np.cumsum(M_list)]).astype(int)

    nc = bacc.Bacc("TRN2", target_bir_lowering=False, debug=False,
                   num_devices=NCORES)

    t_xT = nc.dram_tensor("xT", [D, NPC], BF16, kind="ExternalInput")
    t_wah = nc.dram_tensor("W_ah", [D, D], BF16, kind="ExternalInput")
    t_bah = nc.dram_tensor("bah", [P, KT], F32, kind="ExternalInput")
    t_wall = nc.dram_tensor("W_all", [D, 3 * D], BF16, kind="ExternalInput")
    t_waa = nc.dram_tensor("Waa", [D, 24], BF16, kind="ExternalInput")
    t_esrc = nc.dram_tensor("esrc16", [P, TOT_M * 8], I16,
                            kind="ExternalInput")
    t_edst = nc.dram_tensor("edst", [P, TOT_M], F32, kind="ExternalInput")
    t_edstT = nc.dram_tensor("edstT", [P, TOT_M * P], BF16,
                             kind="ExternalInput")
    t_iota = nc.dram_tensor("iota", [P, P], F32, kind="ExternalInput")
    t_iotac = nc.dram_tensor("iotac", [P, 1], F32, kind="ExternalInput")
    t_ident = nc.dram_tensor("identb", [P, P], BF16, kind="ExternalInput")
    if has_b:
        t_bcat = nc.dram_tensor("bcat_bc", [P, 3 * D], F32,
                                kind="ExternalInput")
    t_wout = nc.dram_tensor("W_out", [3 * D, D], BF16, kind="ExternalInput")
    if has_bout:
        t_bout = nc.dram_tensor("bout_bc", [P, D], F32, kind="ExternalInput")
    t_out = nc.dram_tensor("out_slice", [NPC, D], F32, kind="ExternalOutput")

    mm = nc.tensor.matmul
    eq = mybir.AluOpType.is_equal
    ADD = mybir.AluOpType.add
    MUL = mybir.AluOpType.mult
    AF = mybir.ActivationFunctionType

    with tile.TileContext(nc) as tc:
        with (
            tc.tile_pool(name="consts", bufs=2) as cp,
            tc.tile_pool(name="nemb", bufs=2) as nep,
            tc.tile_pool(name="dram", bufs=2, space="DRAM") as dp,
        ):
            for rep in range(reps):
                iota_sb = cp.tile([P, P], F32)
                nc.sync.dma_start(iota_sb[:], t_iota.ap())
                iotac_sb = cp.tile([P, 1], F32)
                nc.sync.dma_start(iotac_sb[:], t_iotac.ap())
                ident_sb = cp.tile([P, P], BF16)
                nc.sync.dma_start(ident_sb[:], t_ident.ap())
                esrc_sb = cp.tile([P, TOT_M * 8], I16)
                nc.sync.dma_start(esrc_sb[:], t_esrc.ap())
                edst_sb = cp.tile([P, TOT_M], F32)
                nc.sync.dma_start(edst_sb[:], t_edst.ap())
                bah_sb = cp.tile([P, KT], F32)
                nc.sync.dma_start(bah_sb[:], t_bah.ap())
                if has_b:
                    bcat_sb = cp.tile([P, 3 * D], F32)
                    nc.sync.dma_start(bcat_sb[:], t_bcat.ap())
                if has_bout:
                    bout_sb = cp.tile([P, D], F32)
                    nc.sync.dma_start(bout_sb[:], t_bout.ap())
                asad_sb = cp.tile([P, NTILES * 24], F32)
                nc.gpsimd.memset(asad_sb[:], 0.0)
                asad_bf = cp.tile([P, NTILES * 24], BF16)
                nodeemb = nep.tile([P, NTILES * 3 * D], BF16)

                hext = dp.tile([NPC, FROWP], BF16, name="hext")
                hfull = dp.tile([NCORES * NPC, FROWP], BF16,
                                addr_space="Shared", name="hfull")

                # ---------------- phase 1: dense matmuls ----------------
                with tc.tile_pool(name="ph1x1", bufs=1) as xp1:
                    x1sb = [xp1.tile([P, NPC], BF16, name=f"x1_{k}")
                            for k in range(KT)]
                    with (
                        tc.tile_pool(name="ph1a", bufs=1) as wpa,
                        tc.tile_pool(name="px1", bufs=1, space="PSUM") as px1,
                    ):
                        xsb, wah = [], []
                        for k in range(KT):
                            xk = wpa.tile([P, NPC], BF16, name=f"xsb{k}")
                            nc.sync.dma_start(
                                xk[:], t_xT.ap()[k * P : (k + 1) * P, :])
                            xsb.append(xk)
                            wk = wpa.tile([P, D], BF16, name=f"wah{k}")
                            nc.sync.dma_start(
                                wk[:], t_wah.ap()[k * P : (k + 1) * P, :])
                            wah.append(wk)

                        # x1T = relu(x @ W_ah + b), feat-major [768, 1250]
                        nch = [(0, 512), (512, 512), (1024, NPC - 1024)]
                        for jp in range(KT // 2):
                            pss = [px1.tile([P, 512], F32, space="PSUM",
                                            name=f"psx1_{q}")
                                   for q in range(6)]
                            for k in range(KT):
                                for j2 in range(2):
                                    j = jp * 2 + j2
                                    for n, (n0, nw) in enumerate(nch):
                                        mm(out=pss[j2 * 3 + n][:, :nw],
                                           lhsT=wah[k][:, j * P
                                                       : (j + 1) * P],
                                           rhs=xsb[k][:, n0 : n0 + nw],
                                           start=(k == 0),
                                           stop=(k == KT - 1))
                            for j2 in range(2):
                                j = jp * 2 + j2
                                for n, (n0, nw) in enumerate(nch):
                                    nc.scalar.activation(
                                        x1sb[j][:, n0 : n0 + nw],
                                        pss[j2 * 3 + n][:, :nw], AF.Relu,
                                        bias=bah_sb[:, j : j + 1])

                    with (
                        tc.tile_pool(name="ph1b", bufs=1) as wpb,
                        tc.tile_pool(name="ph1st", bufs=3) as sp1,
                        tc.tile_pool(name="ph", bufs=1, space="PSUM") as phh,
                    ):
                        wall, waa = [], []
                        for k in range(KT):
                            ak = wpb.tile([P, 3 * D], BF16, name=f"wall{k}")
                            nc.sync.dma_start(
                                ak[:], t_wall.ap()[k * P : (k + 1) * P, :])
                            wall.append(ak)
                            bk = wpb.tile([P, 24], BF16, name=f"waa{k}")
                            nc.sync.dma_start(
                                bk[:], t_waa.ap()[k * P : (k + 1) * P, :])
                            waa.append(bk)

                        # h_l = x1 @ W_l for l=0..2 and asad = x1 @ Waa
                        for t in range(NTILES):
                            m0 = t * P
                            mw = min(P, NPC - m0)
                            hps = [phh.tile([P, 384], F32, space="PSUM",
                                            name=f"psh{q}")
                                   for q in range(6)]
                            aps = phh.tile([P, 24], F32, space="PSUM",
                                           name="psasad")
                            for k in range(KT):
                                for q in range(6):
                                    mm(out=hps[q][:mw, :],
                                       lhsT=x1sb[k][:, m0 : m0 + mw],
                                       rhs=wall[k][:, q * 384 : q * 384 + 384],
                                       start=(k == 0), stop=(k == KT - 1))
                                mm(out=aps[:mw, :],
                                   lhsT=x1sb[k][:, m0 : m0 + mw],
                                   rhs=waa[k][:],
                                   start=(k == 0), stop=(k == KT - 1))
                            nc.vector.tensor_copy(
                                asad_sb[:mw, t * 24 : t * 24 + 24],
                                aps[:mw, :])
                            st = sp1.tile([P, FROWP], BF16, name="hstage")
                            nc.gpsimd.memset(st[:, FROW:FROWP], 0.0)
                            for l in range(3):
                                for hh in range(2):
                                    nc.scalar.activation(
                                        st[:mw, l * FEXT + hh * 384
                                           : l * FEXT + hh * 384 + 384],
                                        hps[l * 2 + hh][:mw, :], AF.Copy)
                                nc.vector.tensor_copy(
                                    st[:mw, l * FEXT + D : l * FEXT + D + H],
                                    asad_sb[:mw, t * 24 + l * 8
                                            : t * 24 + l * 8 + 4])
                            nc.sync.dma_start(
                                hext[m0 : m0 + mw, :], st[:mw, :])
                        nc.vector.tensor_copy(asad_bf[:], asad_sb[:])
                        if not no_cc:
                            nc.gpsimd.collective_compute(
                                "AllGather", mybir.AluOpType.bypass,
                                replica_groups=[list(range(NCORES))],
                                ins=[hext[:].opt()],
                                outs=[hfull[:].opt()],
                            )

                # -------------- phase 3: gather + aggregate --------------
                if 3 in phases:
                    with (
                        tc.tile_pool(name="edtp", bufs=2) as edtp,
                        tc.tile_pool(name="hgp", bufs=2) as hgp,
                        tc.tile_pool(name="sgp", bufs=3) as sgp,
                        tc.tile_pool(name="smallp", bufs=4) as smp,
                        tc.tile_pool(name="psA", bufs=1, space="PSUM") as psA,
                        tc.tile_pool(name="psB", bufs=2, space="PSUM") as psB,
                    ):
                        nsc = 0  # alpha-scale op round-robin counter
                        for t in range(NTILES):
                            Mt = M_list[t]
                            Moff = int(M_off[t])
                            mw = min(P, NPC - t * P)
                            edstT_sb = edtp.tile([P, Mt * P], BF16,
                                                 name="edtw")
                            nc.sync.dma_start(
                                edstT_sb[:],
                                t_edstT.ap()[:, Moff * P : (Moff + Mt) * P])
                            aggs = [psA.tile([P, FEXT], F32, space="PSUM",
                                             name=f"agg{l}")
                                    for l in range(3)]
                            for c0 in range(0, Mt, HG):
                                gl = min(HG, Mt - c0)
                                gi0 = c0  # first chunk index in tile
                                hg = hgp.tile([P, gl * FROWP], BF16,
                                              name="hg")
                                nc.gpsimd.dma_gather(
                                    out_ap=hg[:].rearrange(
                                        "p (m f) -> p m f", f=FROWP),
                                    in_ap=hfull[:],
                                    idxs_ap=esrc_sb[:, (Moff + c0) * 8
                                                    : (Moff + c0 + gl) * 8],
                                    num_idxs=gl * P,
                                    num_idxs_reg=gl * P,
                                    elem_size=FROWP,
                                )
                                st_all = sgp.tile([P, gl * P], BF16,
                                                  name="st_all")
                                nc.vector.tensor_scalar(
                                    out=st_all[:],
                                    in0=edstT_sb[:, c0 * P : (c0 + gl) * P],
                                    scalar1=iotac_sb[:, 0:1], scalar2=None,
                                    op0=eq)
                                s_all = sgp.tile([P, gl * P], BF16,
                                                 name="s_all")
                                nc.vector.tensor_tensor(
                                    out=s_all[:].rearrange(
                                        "p (m n) -> p m n", n=P),
                                    in0=iota_sb[:].unsqueeze(1).to_broadcast(
                                        [P, gl, P]),
                                    in1=edst_sb[:, Moff + c0 : Moff + c0 + gl]
                                    .unsqueeze(2).to_broadcast([P, gl, P]),
                                    op=eq)
                                adp = psB.tile([P, HG * 12], F32,
                                               space="PSUM", name="adp")
                                for m in range(gl):
                                    for l in range(3):
                                        ad_col = t * 24 + l * 8 + 4
                                        o = m * 12 + l * 4
                                        mm(out=adp[:, o : o + 4],
                                           lhsT=st_all[:, m * P
                                                       : (m + 1) * P],
                                           rhs=asad_bf[:, ad_col
                                                       : ad_col + H],
                                           start=True, stop=True)
                                # logits: as[src] + ad[dst], leaky relu, exp
                                hg4 = hg[:].rearrange(
                                    "p (m f) -> p m f", f=FROWP)
                                t_all = smp.tile([P, gl * 12], F32,
                                                 name="t_all")
                                t4 = t_all[:].rearrange(
                                    "p (m g) -> p m g", g=12)
                                adp4 = adp[:, : gl * 12].rearrange(
                                    "p (m g) -> p m g", g=12)
                                for l in range(3):
                                    nc.vector.tensor_tensor(
                                        out=t4[:, :, l * H : (l + 1) * H],
                                        in0=hg4[:, :, l * FEXT + D
                                                : l * FEXT + D + H],
                                        in1=adp4[:, :, l * H : (l + 1) * H],
                                        op=ADD)
                                u_all = smp.tile([P, gl * 12], F32,
                                                 name="u_all")
                                nc.vector.tensor_scalar_mul(
                                    u_all[:], t_all[:], NEG_SLOPE)
                                nc.vector.tensor_tensor(
                                    out=t_all[:], in0=t_all[:], in1=u_all[:],
                                    op=mybir.AluOpType.max)
                                p_all = smp.tile([P, gl * 12], F32,
                                                 name="p_all")
                                nc.scalar.activation(p_all[:], t_all[:],
                                                     AF.Exp)
                                # write exp into the gathered rows' as slots
                                p4 = p_all[:].rearrange(
                                    "p (m g) -> p m g", g=12)
                                for l in range(3):
                                    nc.vector.tensor_copy(
                                        hg4[:, :, l * FEXT + D
                                            : l * FEXT + D + H],
                                        p4[:, :, l * H : (l + 1) * H])
                                # alpha-scale h blocks, split DVE / ACT
                                for m in range(gl):
                                    for l in range(3):
                                        base = m * FROWP + l * FEXT
                                        for h in range(H):
                                            pc = m * 12 + l * 4 + h
                                            sl = slice(base + h * C,
                                                       base + (h + 1) * C)
                                            nsc += 1
                                            if nsc % 3 != 0:
                                                nc.vector.tensor_scalar(
                                                    out=hg[:, sl],
                                                    in0=hg[:, sl],
                                                    scalar1=p_all[:, pc
                                                                  : pc + 1],
                                                    scalar2=None, op0=MUL)
                                            else:
                                                nc.scalar.activation(
                                                    hg[:, sl], hg[:, sl],
                                                    AF.Copy,
                                                    scale=p_all[:, pc
                                                                : pc + 1])
                                for m in range(gl):
                                    gi = gi0 + m
                                    for l in range(3):
                                        base = m * FROWP + l * FEXT
                                        mm(out=aggs[l][:, 0:512],
                                           lhsT=s_all[:, m * P : (m + 1) * P],
                                           rhs=hg[:, base : base + 512],
                                           start=(gi == 0),
                                           stop=(gi == Mt - 1))
                                        mm(out=aggs[l][:, 512:FEXT],
                                           lhsT=s_all[:, m * P : (m + 1) * P],
                                           rhs=hg[:, base + 512
                                                  : base + FEXT],
                                           start=(gi == 0),
                                           stop=(gi == Mt - 1))
                            for l in range(3):
                                invs = smp.tile([P, H], F32, name="invs")
                                nc.vector.tensor_scalar_add(
                                    invs[:], aggs[l][:, D : D + H], 1e-16)
                                nc.vector.reciprocal(invs[:], invs[:])
                                ne0 = t * 3 * D + l * D
                                if has_b:
                                    for h in range(H):
                                        nc.vector.tensor_scalar(
                                            out=nodeemb[:, ne0 + h * C
                                                        : ne0 + (h + 1) * C],
                                            in0=aggs[l][:, h * C
                                                        : (h + 1) * C],
                                            scalar1=invs[:, h : h + 1],
                                            scalar2=None, op0=MUL)
                                    nc.vector.tensor_tensor(
                                        out=nodeemb[:, ne0 : ne0 + D],
                                        in0=nodeemb[:, ne0 : ne0 + D],
                                        in1=bcat_sb[:, l * D : (l + 1) * D],
                                        op=ADD)
                                    nc.scalar.activation(
                                        nodeemb[:, ne0 : ne0 + D],
                                        nodeemb[:, ne0 : ne0 + D], AF.Relu)
                                else:
                                    for h in range(H):
                                        nc.scalar.activation(
                                            nodeemb[:, ne0 + h * C
                                                    : ne0 + (h + 1) * C],
                                            aggs[l][:, h * C : (h + 1) * C],
                                            AF.Relu,
                                            scale=invs[:, h : h + 1])

                # -------------- phase 4: output projection --------------
                if 4 in phases:
                    with (
                        tc.tile_pool(name="ph4w", bufs=1) as wp4,
                        tc.tile_pool(name="ph4s", bufs=2) as sp4,
                        tc.tile_pool(name="pT4", bufs=2, space="PSUM") as pT4,
                        tc.tile_pool(name="pO4", bufs=1, space="PSUM") as pO4,
                    ):
                        wout = []
                        for k in range(3 * KT):
                            wk = wp4.tile([P, D], BF16, name=f"wout{k}")
                            nc.sync.dma_start(
                                wk[:], t_wout.ap()[k * P : (k + 1) * P, :])
                            wout.append(wk)
                        for t in range(NTILES):
                            mw = min(P, NPC - t * P)
                            tsbs = []
                            for hf in range(2):
                                tp = pT4.tile([P, 9 * P], BF16, space="PSUM",
                                              name="tps")
                                for k9 in range(9):
                                    k = hf * 9 + k9
                                    nc.tensor.transpose(
                                        out=tp[:, k9 * P : (k9 + 1) * P],
                                        in_=nodeemb[:, t * 3 * D + k * P
                                                    : t * 3 * D
                                                    + (k + 1) * P],
                                        identity=ident_sb[:])
                                tsb = sp4.tile([P, 9 * P], BF16, name="tsb",
                                               bufs=4)
                                nc.vector.tensor_copy(tsb[:], tp[:])
                                tsbs.append(tsb)
                            op = pO4.tile([P, D], F32, space="PSUM",
                                          name="ops")
                            for k in range(3 * KT):
                                lh = tsbs[k // 9][:, (k % 9) * P
                                                  : (k % 9 + 1) * P]
                                for (n0, nw) in [(0, 512), (512, 256)]:
                                    mm(out=op[:, n0 : n0 + nw],
                                       lhsT=lh,
                                       rhs=wout[k][:, n0 : n0 + nw],
                                       start=(k == 0), stop=(k == 3 * KT - 1))
                            stg = sp4.tile([P, D], F32, name="ostage")
                            if has_bout:
                                nc.vector.tensor_tensor(out=stg[:], in0=op[:],
                                                        in1=bout_sb[:],
                                                        op=ADD)
                                nc.scalar.activation(stg[:], stg[:], AF.Relu)
                            else:
                                nc.scalar.activation(stg[:], op[:], AF.Relu)
                            nc.sync.dma_start(
                                t_out.ap()[t * P : t * P + mw, :],
                                stg[:mw, :])

    nc.compile()
    return nc


def _host_prep(inputs):
    x = np.asarray(inputs["x"], dtype=np.float32)
    edge_index = np.asarray(inputs["edge_index"], dtype=np.int32)
    W_ah = np.asarray(inputs["W_ah"], dtype=np.float32)
    b_ah = np.asarray(inputs["b_ah"], dtype=np.float32)
    W_out = np.asarray(inputs["W_out"], dtype=np.float32)
    b_out = np.asarray(inputs["b_out"], dtype=np.float32)
    Ws = [np.asarray(inputs[f"W{i}"], dtype=np.float32) for i in range(3)]
    asrcs = [np.asarray(inputs[f"a_src{i}"], dtype=np.float32)
             for i in range(3)]
    adsts = [np.asarray(inputs[f"a_dst{i}"], dtype=np.float32)
             for i in range(3)]
    bs = [np.asarray(inputs[f"b{i}"], dtype=np.float32) for i in range(3)]

    esrc16, edst, edstT, M_list = _prep_edges(edge_index)

    W_all = np.ascontiguousarray(np.concatenate(Ws, axis=1))
    Waa = np.zeros((D, 24), dtype=np.float32)
    for l in range(3):
        Amat_s = np.zeros((D, H), dtype=np.float32)
        Amat_d = np.zeros((D, H), dtype=np.float32)
        for h in range(H):
            Amat_s[h * C : (h + 1) * C, h] = asrcs[l][h]
            Amat_d[h * C : (h + 1) * C, h] = adsts[l][h]
        Waa[:, l * 8 : l * 8 + 4] = Ws[l] @ Amat_s
        Waa[:, l * 8 + 4 : l * 8 + 8] = Ws[l] @ Amat_d

    bah2 = np.ascontiguousarray(b_ah.reshape(KT, P).T)
    bcat = np.concatenate(bs)
    has_b = bool(np.any(bcat))
    has_bout = bool(np.any(b_out))

    iota = np.broadcast_to(np.arange(P, dtype=np.float32), (P, P)).copy()
    iotac = np.arange(P, dtype=np.float32).reshape(P, 1).copy()
    identb = np.eye(P, dtype=NPBF)

    shared = {
        "W_ah": W_ah.astype(NPBF), "bah": bah2,
        "W_all": W_all.astype(NPBF), "Waa": Waa.astype(NPBF),
        "iota": iota, "iotac": iotac, "identb": identb,
        "W_out": np.ascontiguousarray(W_out).astype(NPBF),
    }
    if has_b:
        shared["bcat_bc"] = np.broadcast_to(bcat, (P, 3 * D)).copy()
    if has_bout:
        shared["bout_bc"] = np.broadcast_to(b_out, (P, D)).copy()

    in_maps = []
    for c in range(NCORES):
        m = dict(shared)
        m["xT"] = np.ascontiguousarray(
            x[c * NPC : (c + 1) * NPC].T).astype(NPBF)
        m["esrc16"] = np.ascontiguousarray(esrc16[c])
        m["edst"] = np.ascontiguousarray(edst[c])
        m["edstT"] = np.ascontiguousarray(edstT[c])
        in_maps.append(m)
    return in_maps, M_list, has_b, has_bout


def run(inputs, trace=False, dbg=False, reps=1):
    in_maps, M_list, has_b, has_bout = _host_prep(inputs)
    key = (M_list, has_b, has_bout, dbg, reps)
    if key not in _cache:
        _cache[key] = _build(M_list, has_b, has_bout, dbg=dbg,
                             reps=reps)
    nc = _cache[key]
    res = run_bass_kernel_spmd(nc, in_maps, core_ids=list(range(NCORES)),
                               trace=trace)
    out = np.concatenate([res.results[c]["out_slice"]
                          for c in range(NCORES)], axis=0)
    return out, res


def kernel(**inputs) -> np.ndarray:
    out, _ = run(inputs, trace=False)
    return out
